# revision 17
# baseline (speedup 1.0000x reference)
"""Local (sliding-window) self-attention Trainium2 kernel, 8-core SPMD.

Problem: nn_LocalSelfAttention — S=4096, B=2, E=768, H=12, D=64, window
overlap w=256 (band of 2w+1=513 keys per query), key padding mask.

Sharding: batch*head parallel. Core c owns batch c//4 and heads
3*(c%4) .. 3*(c%4)+3.  No cross-core communication.

v4 design (v3 + pipeline smoothing):
  Projections: wst stationaries ([Wq0|Wq1],[Wk0|Wk1],[Wq2|Wk2]) produce
  qT/kT feature-major; token-stationary MMs produce V natural,
  masked+ones-column -> va = [V*m | m].
  Scores are key-block-major.  For key block kc the valid q-window is
  chunks [kc-2, kc+3), split A = chunks [kc-2, kc+2) (<=512 wide,
  PSUM-bank aligned) and B = the freshest chunk kc+2 (128 wide).
  h0 (rows 0:64) and h1 (64:128) co-execute via disjoint PE row groups
  (different PSUM banks); h2's A runs on rows 0:64 and co-executes with
  h1's B (rows 64:128, own bank slot) which retires before h0's B can
  start, so the shared psB bank is never written concurrently.
  exp: 3 ACT ops per kc into one [128, 3, 640] bf16 pe tile; band-edge
  triangular masks multiply the first A chunk (triu) and the B chunk
  (tril) on DVE, emitted after the projection copies so they never
  block psP reuse.
  PV: lhsT = pe[keys, q-chunk] slices, rhs = va[:,kc,h,:] (65 cols incl
  ones), accumulating po[q, 3*65] at a 3-tt trail so it never waits on
  same-iteration exps.  Column 64 of each head's slot is the softmax
  denominator; the final divide happens on the host.
  Pipeline smoothing: valT slices prefetched 2 periods ahead; 8 warmup
  matmuls keep the PE busy during the initial DMA so the HAM clock gate
  reaches 8/8 early; score/PV matmuls are interleaved between the
  single-buffered psP projection groups to hide their copy-out waits;
  output DMA rides the ACT hwdge queue so it can't delay the k2s copy.
"""

import sys

sys.path.insert(0, "/opt/trn_rl_repo")

import numpy as np

S = 4096
B = 2
E = 768
H = 12
D = 64
WO = 256  # one-sided window (w)
NCORES = 8
HPC = 3  # heads per core
NT = S // 128  # 32 token chunks
KC = E // 128  # 6 contraction chunks
TT4 = S // 512  # 8 projection token tiles

_CACHE = {}


def _build_program(
    with_qk_bias=False,
    use_scalar_dma=True,
    use_warmup=False,
    use_big_valt_dma=True,
    use_b_coexec=False,
):
    import concourse.bacc as bacc
    import concourse.tile as tile
    from concourse import mybir

    BF = mybir.dt.bfloat16
    F32 = mybir.dt.float32
    AF = mybir.ActivationFunctionType

    nc = bacc.Bacc()

    # host pre-rearranged: valT[p, kc, s], wst[p, kc, st, j]
    valT = nc.declare_dram_parameter("valT", [128, KC * S], BF, isOutput=False)
    wst = nc.declare_dram_parameter("wst", [128, KC * 3 * 128], BF, isOutput=False)
    bst = nc.declare_dram_parameter("bst", [128, 3], F32, isOutput=False)
    wv = nc.declare_dram_parameter("wv", [E, HPC * D], BF, isOutput=False)
    m32 = nc.declare_dram_parameter("m32", [128, NT], F32, isOutput=False)
    m16 = nc.declare_dram_parameter("m16", [128, NT * HPC], BF, isOutput=False)
    # tri3[:, 0] = triu replicated x3 heads, tri3[:, 1] = tril x3
    tri3 = nc.declare_dram_parameter("tri3", [128, 2 * HPC * 128], BF, isOutput=False)
    # numerator columns h*65 .. h*65+63, denominator at h*65+64
    outp = nc.declare_dram_parameter("out", [S, HPC * (D + 1)], F32, isOutput=True)

    with tile.TileContext(nc) as tc:
        with (
            tc.tile_pool(name="consts", bufs=1) as consts,
            tc.tile_pool(name="big", bufs=1) as big,
            tc.tile_pool(name="pep", bufs=8) as pep,
            tc.tile_pool(name="psSA", bufs=2, space="PSUM") as psSA,
            tc.tile_pool(name="psA2", bufs=1, space="PSUM") as psA2,
            tc.tile_pool(name="psB", bufs=1, space="PSUM") as psB,
            tc.tile_pool(name="psO", bufs=1, space="PSUM") as psO,
            tc.tile_pool(name="psP", bufs=1, space="PSUM") as psP,
        ):
            # ---- constants.  Everything needed early rides the two
            # HWDGE queues (sync/scalar): the gpsimd software DGE takes
            # ~10us of ucode boot before it moves its first byte, which
            # stalled the whole prologue when the weights lived there. ----
            wst_t = consts.tile([128, KC, 3, 128], BF)
            wv_t = consts.tile([128, KC, HPC * D], BF)
            bst_t = consts.tile([128, 3], F32)
            m32_t = consts.tile([128, NT], F32)
            m16_t = consts.tile([128, NT, HPC], BF)
            tri_t = consts.tile([128, 2, HPC, 128], BF)

            def emit_const_dmas():
                # batched host-prearranged transfers; the kc=0 pieces land
                # first so the warmups and first projection start ASAP
                wstv = wst[:, :].rearrange("p (kc st j) -> p kc st j", kc=KC, st=3)
                valv = valT[:, :].rearrange("p (kc s) -> p kc s", kc=KC)
                nc.sync.dma_start(out=wst_t[:, 0, :, :], in_=wstv[:, 0, :, :])
                nc.sync.dma_start(out=vTT[:, 0, 0:512], in_=valv[:, 0, 0:512])
                nc.sync.dma_start(out=wst_t[:, 1:KC, :, :], in_=wstv[:, 1:KC, :, :])
                nc.scalar.dma_start(
                    out=vTT[:, 1:KC, 0:512], in_=valv[:, 1:KC, 0:512]
                )
                nc.scalar.dma_start(
                    out=wv_t, in_=wv[:, :].rearrange("(kc p) n -> p kc n", p=128)
                )
                nc.scalar.dma_start(out=bst_t, in_=bst[:, :])
                nc.scalar.dma_start(out=m32_t, in_=m32[:, :])
                nc.scalar.dma_start(
                    out=m16_t, in_=m16[:, :].rearrange("p (t h) -> p t h", h=HPC)
                )
                nc.scalar.dma_start(
                    out=tri_t,
                    in_=tri3[:, :].rearrange("p (e h j) -> p e h j", e=2, h=HPC),
                )

            # ---- val^T in SBUF as [128, kc, S] ----
            vTT = big.tile([128, KC, S], BF, tag="vTT", name="vTT")
            vT = [vTT[:, kc, :] for kc in range(KC)]

            def emit_valT_dma(t4):
                sl = slice(t4 * 512, (t4 + 1) * 512)
                nc.sync.dma_start(
                    out=vTT[:, :, sl],
                    in_=valT[:, :].rearrange("p (kc s) -> p kc s", kc=KC)[:, :, sl],
                )

            # persistent projection outputs
            qq = big.tile([128, S], BF, tag="qq")  # qT h0 @0:64, qT h1 @64:128
            kk = big.tile([128, S], BF, tag="kk")  # kT h0 @0:64, kT h1 @64:128
            qk2 = big.tile([128, S], BF, tag="qk2")  # qT h2 @0:64, kT h2 @64:128
            k2s = big.tile([128, S], BF, tag="k2s")  # kT h2 copied to @0:64
            if not use_b_coexec:
                q1s = big.tile([128, S], BF, tag="q1s")  # qT h1 copied to @0:64
                k1s = big.tile([128, S], BF, tag="k1s")  # kT h1 copied to @0:64
            st_dst = [qq, kk, qk2]
            # [V*m | m] for all heads: [128, tt, h, 65]
            va = big.tile([128, NT, HPC, D + 1], BF, tag="va", name="va")
            # staged output [q-part, qc, 3*65]
            ost = big.tile([128, NT, HPC * (D + 1)], F32, tag="ost", name="ost")

            emit_const_dmas()
            emit_valT_dma(1)

            # preload the ACT exp table during the prologue so the first
            # real exp doesn't pay the ~1.3us ACT_TABLE_LOAD
            actw = consts.tile([128, 8], BF)
            nc.scalar.activation(actw, tri_t[:, 0, 0, 0:8], AF.Exp)

            def emit_warm():
                # filler matmul into the psSA banks (idle until tt=4): keeps
                # the PE busy across early psP copy-out waits and drives the
                # HAM clock gate to 8/8 during the prologue
                wsa = psSA.tile([128, 2, 512], F32, tag="sa", name="wsa")
                nc.tensor.matmul(
                    wsa[:, 0, 0:384],
                    lhsT=wst_t[:, 0, 0, :],
                    rhs=wst_t[:, 0, :, :],
                    start=True,
                    stop=True,
                )

            emit_warm()
            emit_warm()

            def emit_proj_qk_st(t4, st):
                sl = slice(t4 * 512, (t4 + 1) * 512)
                ps = psP.tile([128, 512], F32, tag="proj")
                for kc in range(KC):
                    nc.tensor.matmul(
                        ps,
                        lhsT=wst_t[:, kc, st, :],
                        rhs=vT[kc][:, sl],
                        start=(kc == 0),
                        stop=(kc == KC - 1),
                    )
                if with_qk_bias:
                    nc.vector.tensor_scalar_add(
                        st_dst[st][:, sl], in0=ps, scalar1=bst_t[:, st : st + 1]
                    )
                else:
                    nc.vector.tensor_copy(st_dst[st][:, sl], ps)

            def emit_k2s_copy(t4):
                # h2's k to the low partition half so its A matmul (and
                # B-h2) can run on rows 0:64 against qT2 which lives there
                sl = slice(t4 * 512, (t4 + 1) * 512)
                # on the SP queue: a trigger whose wait is unsatisfied
                # blocks everything behind it, and SP carries no compute
                nc.sync.dma_start(out=k2s[0:64, sl], in_=qk2[64:128, sl])
                if not use_b_coexec:
                    nc.sync.dma_start(out=q1s[0:64, sl], in_=qq[64:128, sl])
                    nc.sync.dma_start(out=k1s[0:64, sl], in_=kk[64:128, sl])

            def emit_proj_v(tt):
                sl = slice(tt * 128, (tt + 1) * 128)
                ps = psP.tile([128, HPC * D], F32, tag="proj")
                for kc in range(KC):
                    nc.tensor.matmul(
                        ps,
                        lhsT=vT[kc][:, sl],
                        rhs=wv_t[:, kc, :],
                        start=(kc == 0),
                        stop=(kc == KC - 1),
                    )
                nc.vector.tensor_scalar_mul(
                    va[:, tt, :, 0:D],
                    in0=ps[:, :].rearrange("p (h d) -> p h d", h=HPC),
                    scalar1=m32_t[:, tt : tt + 1],
                )
                nc.vector.tensor_copy(va[:, tt, :, D], m16_t[:, tt, :])

            pe_of = {}  # kc -> [128, 3, 640] bf16 tile (h0, h1, h2 windows)
            sa_of = {}

            def awin(kc):
                lo = max(0, kc - 2)
                ahi = min(kc + 2, NT)
                return lo, (ahi - lo) * 128  # A starts at chunk lo, wA wide

            def emit_scores_pair(kc):
                lo, wA = awin(kc)
                asl = slice(lo * 128, lo * 128 + wA)
                ktb = slice(kc * 128, (kc + 1) * 128)
                sa = psSA.tile([128, 2, 512], F32, tag="sa")
                sa_of[kc] = sa
                # h0/h1 A-windows co-execute on disjoint PE row halves;
                # they write different PSUM banks (slots 0/1 of sa).
                nc.tensor.matmul(
                    sa[:, 0, 0:wA], lhsT=kk[0:64, ktb], rhs=qq[0:64, asl],
                    start=True, stop=True,
                )
                nc.tensor.matmul(
                    sa[:, 1, 0:wA], lhsT=kk[64:128, ktb], rhs=qq[64:128, asl],
                    start=True, stop=True,
                )

            def emit_scores_rest(kc):
                lo, wA = awin(kc)
                has_b = kc + 2 < NT
                asl = slice(lo * 128, lo * 128 + wA)
                ktb = slice(kc * 128, (kc + 1) * 128)
                sa = sa_of.pop(kc)
                sb = None
                if has_b:
                    bsl = slice((kc + 2) * 128, (kc + 3) * 128)
                    sb = psB.tile([128, HPC, 128], F32, tag="sb")
                    if use_b_coexec:
                        # h1's B first, on rows 64:128: it starts strictly
                        # before sa2 (in-order starts) and its 128 columns
                        # retire while sa2 (>=256 cols) still occupies rows
                        # 0:64, which is what blocks B-h0 -> the shared psB
                        # bank is never written by two in-flight matmuls.
                        nc.tensor.matmul(
                            sb[:, 1, :], lhsT=kk[64:128, ktb], rhs=qq[64:128, bsl],
                            start=True, stop=True,
                        )
                # h2's A on rows 0:64 (its own bank); co-executes with B-h1
                sa2 = psA2.tile([128, 512], F32, tag="sa2")
                nc.tensor.matmul(
                    sa2[:, 0:wA],
                    lhsT=k2s[0:64, ktb],
                    rhs=qk2[0:64, asl],
                    start=True,
                    stop=True,
                )
                if has_b:
                    nc.tensor.matmul(
                        sb[:, 0, :], lhsT=kk[0:64, ktb], rhs=qq[0:64, bsl],
                        start=True, stop=True,
                    )
                    if not use_b_coexec:
                        nc.tensor.matmul(
                            sb[:, 1, :], lhsT=k1s[0:64, ktb], rhs=q1s[0:64, bsl],
                            start=True, stop=True,
                        )
                    nc.tensor.matmul(
                        sb[:, 2, :], lhsT=k2s[0:64, ktb], rhs=qk2[0:64, bsl],
                        start=True, stop=True,
                    )
                # exp -> pe tile; A2 first (psA2 is single-buffered, so
                # the next kc's h2 matmul waits on this read), then the
                # A-pair, then B last (it waits on the late B matmuls).
                pe = pep.tile([128, HPC, 640], BF, tag="pe")
                pe_of[kc] = pe
                nc.scalar.activation(pe[:, 2, 0:wA], sa2[:, 0:wA], AF.Exp)
                nc.scalar.activation(pe[:, 0:2, 0:wA], sa[:, :, 0:wA], AF.Exp)
                if has_b:
                    nc.scalar.activation(
                        pe[:, :, wA : wA + 128], sb[:, :, :], AF.Exp
                    )

            def emit_scores_masks(kc):
                # emitted after the projection copies so the DVE queue
                # never delays psP reuse behind an exp-dependent multiply
                lo, wA = awin(kc)
                pe = pe_of[kc]
                if kc + 2 < NT:
                    # hi edge (chunk kc+2): q <= key+256 -> tril, all heads
                    nc.vector.tensor_mul(
                        pe[:, :, wA : wA + 128],
                        pe[:, :, wA : wA + 128],
                        tri_t[:, 1, :, :],
                    )
                if kc - 2 >= 0:
                    # lo edge (chunk kc-2): q >= key-256 -> triu, all heads
                    nc.vector.tensor_mul(
                        pe[:, :, 0:128], pe[:, :, 0:128], tri_t[:, 0, :, :]
                    )

            po_of = {}

            def emit_pv(qc, hs):
                kcs = [k for k in range(qc - 2, qc + 3) if 0 <= k < NT]
                if 0 in hs:
                    po_of[qc] = psO.tile([128, 256], F32, tag="po", name="po")
                po = po_of[qc]
                for h in hs:
                    for j, kc in enumerate(kcs):
                        lo, wA = awin(kc)
                        off = (qc - lo) * 128
                        nc.tensor.matmul(
                            po[:, h * (D + 1) : (h + 1) * (D + 1)],
                            lhsT=pe_of[kc][:, h, off : off + 128],
                            rhs=va[:, kc, h, :],
                            start=(j == 0),
                            stop=(j == len(kcs) - 1),
                        )
                if HPC - 1 in hs:
                    po_of.pop(qc)
                    nc.vector.tensor_copy(ost[:, qc, :], po[:, 0 : HPC * (D + 1)])

            def emit_out_dma(qc0, n):
                eng = nc.sync if use_scalar_dma else nc.gpsimd
                eng.dma_start(
                    out=outp[qc0 * 128 : (qc0 + n) * 128, :].rearrange(
                        "(t p) c -> p t c", p=128
                    ),
                    in_=ost[:, qc0 : qc0 + n, :],
                )

            # pipelined emission: valT slices land 2 periods early, proj
            # stays ahead of scores by 4 chunks, PV trails scores by 3 so
            # its accumulation never waits on same-iteration exps.  Score
            # and PV matmuls are interleaved between the single-buffered
            # psP projection groups to hide their copy-out waits.
            for tt in range(NT + 7):
                r = tt % 4
                t4 = tt // 4
                qc = tt - 7
                has_pv = 7 <= tt < NT + 7
                if r == 1 and tt < NT and t4 + 2 < TT4:
                    emit_valT_dma(t4 + 2)
                # scores A-pair first: covers the previous proj_v group's
                # psP copy-out before st0 needs the buffer
                if 4 <= tt < NT + 4:
                    emit_scores_pair(tt - 4)
                if r == 0 and tt < NT:
                    emit_proj_qk_st(t4, 0)
                    if tt == 0:
                        emit_warm()
                if 4 <= tt < NT + 4:
                    emit_scores_rest(tt - 4)
                if r == 0 and tt < NT:
                    emit_proj_qk_st(t4, 1)
                    if tt == 0:
                        emit_warm()
                if has_pv:
                    emit_pv(qc, (0, 1) if r == 0 and tt < NT else (0, 1, 2))
                if r == 0 and tt < NT:
                    emit_proj_qk_st(t4, 2)
                    emit_k2s_copy(t4)
                    if tt == 0:
                        emit_warm()
                    if has_pv:
                        emit_pv(qc, (2,))
                if has_pv:
                    if qc % 4 == 3 and qc < NT - 4:
                        emit_out_dma(qc - 3, 4)
                    elif qc >= NT - 4:
                        emit_out_dma(qc, 1)
                if tt < NT:
                    emit_proj_v(tt)
                    if tt < 4:
                        emit_warm()
                    if tt in (1, 2):
                        emit_warm()
                if 4 <= tt < NT + 4:
                    emit_scores_masks(tt - 4)

    nc.finalize()
    return nc


def _prep_inputs(val, key_padding_mask, Wq, bq, Wk, bk, Wv, bv):
    from concourse import mybir

    bf16 = mybir.dt.np(mybir.dt.bfloat16)
    scale = 1.0 / np.sqrt(D)
    Wqs = (np.asarray(Wq, np.float32) * scale).astype(np.float32)
    bqs = np.asarray(bq, np.float32) * scale
    Wk = np.asarray(Wk, np.float32)
    bk = np.asarray(bk, np.float32)
    Wv = np.asarray(Wv, np.float32)
    val = np.asarray(val, np.float32)
    kpm = np.asarray(key_padding_mask)

    tri3 = np.zeros((128, 2, HPC, 128), np.float32)
    tri3[:, 0, :, :] = np.triu(np.ones((128, 128), np.float32))[:, None, :]
    tri3[:, 1, :, :] = np.tril(np.ones((128, 128), np.float32))[:, None, :]
    tri3 = np.ascontiguousarray(tri3.reshape(128, 2 * HPC * 128)).astype(bf16)

    in_maps = []
    for c in range(NCORES):
        b = c // 4
        h0 = HPC * (c % 4)
        valT = np.ascontiguousarray(
            val[:, b, :].T.reshape(KC, 128, S).transpose(1, 0, 2).reshape(128, KC * S)
        ).astype(bf16)

        wst = np.empty((E, 3, 128), np.float32)
        bstm = np.empty((128, 3), np.float32)
        for i, (Wmat, bvec) in enumerate(
            [(Wqs, bqs), (Wk, bk)]
        ):  # st0=[q0|q1], st1=[k0|k1]
            wst[:, i, 0:64] = Wmat[h0 * D : (h0 + 1) * D, :].T
            wst[:, i, 64:128] = Wmat[(h0 + 1) * D : (h0 + 2) * D, :].T
            bstm[0:64, i] = bvec[h0 * D : (h0 + 1) * D]
            bstm[64:128, i] = bvec[(h0 + 1) * D : (h0 + 2) * D]
        wst[:, 2, 0:64] = Wqs[(h0 + 2) * D : (h0 + 3) * D, :].T
        wst[:, 2, 64:128] = Wk[(h0 + 2) * D : (h0 + 3) * D, :].T
        bstm[0:64, 2] = bqs[(h0 + 2) * D : (h0 + 3) * D]
        bstm[64:128, 2] = bk[(h0 + 2) * D : (h0 + 3) * D]

        wvm = np.ascontiguousarray(Wv[h0 * D : (h0 + 3) * D, :].T)

        m = (kpm[b] == 0).astype(np.float32)  # 1.0 = valid key
        m32 = np.ascontiguousarray(m.reshape(NT, 128).T)

        wstr = np.ascontiguousarray(
            wst.reshape(KC, 128, 3, 128).transpose(1, 0, 2, 3).reshape(128, KC * 3 * 128)
        )
        in_maps.append(
            {
                "valT": valT,
                "wst": wstr.astype(bf16),
                "bst": np.ascontiguousarray(bstm),
                "wv": wvm.astype(bf16),
                "m32": m32,
                "m16": np.ascontiguousarray(
                    np.repeat(m32[:, :, None], HPC, axis=2).reshape(128, NT * HPC)
                ).astype(bf16),
                "tri3": tri3,
            }
        )
    return in_maps


def kernel(val, key_padding_mask, Wq, bq, Wk, bk, Wv, bv):
    from concourse.bass_utils import run_bass_kernel_spmd

    with_bias = bool(np.any(np.asarray(bq)) or np.any(np.asarray(bk)))
    key = ("nc", with_bias)
    if key not in _CACHE:
        _CACHE[key] = _build_program(with_qk_bias=with_bias)
        _CACHE["nc"] = _CACHE[key]
    nc = _CACHE[key]

    in_maps = _prep_inputs(val, key_padding_mask, Wq, bq, Wk, bk, Wv, bv)
    res = run_bass_kernel_spmd(nc, in_maps, core_ids=list(range(NCORES)))

    out = np.empty((S, B, E), np.float32)
    for c in range(NCORES):
        b = c // 4
        h0 = HPC * (c % 4)
        r = res.results[c]["out"]  # [S, 3*65] f32
        r3 = r.reshape(S, HPC, D + 1)
        out[:, b, h0 * D : (h0 + 3) * D] = (
            r3[:, :, 0:D] / r3[:, :, D : D + 1]
        ).reshape(S, HPC * D)
    return out
